# revision 38
# baseline (speedup 1.0000x reference)
"""Causal self-attention (B=2, T=2048, C=1024, H=16) on 8 trn2 NeuronCores.

Sharding: tensor-parallel over heads. Each core owns 2 heads (128 channels):
  - qkv projection for its 128 q/k/v columns (full x, transposed layout xT)
  - causal attention for its 2 heads x 2 batches
  - output projection rows for its 128 channels -> partial (4096, 1024) output
Host sums the 8 partial outputs (the "all-reduce") and adds bproj once.

All matmul inputs are fp16 (activations + weights; PSUM accumulates fp32):
fp16 moves 1 col/cycle at any width (f32r drops to 1/4 rate under 256 cols)
and halves HBM traffic. Softmax skips max-subtraction (logits ~ N(0,1)).

Structure per 512-token chunk:
  qkv: q,k wide [ch, t] (stationary w-tile, moving x); v natural [t, ch]
       (stationary x-tile, moving w) so no PE transpose is needed; biases are
       folded into the PSUM->SBUF copies on the Pool engine.
  S:   st[k, q] tiles (stationary kT slice, moving qT), exp on ACT into fp16
       est tiles, causal diag masked post-exp on DVE.
  AV:  est [k, q-128] slices as STATIONARY, moving v_aug [k, 65] (64 v cols +
       ones col) -> yt [q, 65] PSUM; col 64 accumulates the softmax
       denominator per-partition, so normalization is one reciprocal + one
       per-partition tensor_scalar (DVE), no broadcast matmuls.
  y:   merged [t, ch] tile -> DMA-transpose (xbar) -> y [ch, t] fp16.
  proj: stationary y-tile, moving Wproj -> PSUM f32 -> DMA straight to DRAM
       (partial sums; bias added on host).
"""

import sys

if "/opt/trn_rl_repo" not in sys.path:
    sys.path.insert(0, "/opt/trn_rl_repo")

import numpy as np

import concourse.bass as bass
import concourse.mybir as mybir
import concourse.tile as tile
from concourse import bacc
from concourse.bass_utils import run_bass_kernel_spmd

# Problem shape (hardcoded per contest contract)
B, T, C, H = 2, 2048, 1024, 16
D = C // H                # 64 head dim
N_CORES = 8
HPC = H // N_CORES        # 2 heads per core
CH = HPC * D              # 128 channels per core
TALL = B * T              # 4096 flattened tokens
NCT = C // 128            # 8 contraction tiles
TCH = 512                 # t-chunk
NCHUNK = TALL // TCH      # 8
NQC = T // 512            # 4 q-chunks per batch
F32 = mybir.dt.float32
F16 = mybir.dt.float16


def build_nc(phases=("qkv", "att", "proj"), repeat=1, cfg=None):
    cfg = cfg or {}
    ST_W = cfg.get("st_w", 1024)
    ST_B = cfg.get("st_bufs", 2)
    MM_B = cfg.get("mm_bufs", 3)
    EST_B = cfg.get("est_bufs", 28)
    YT_ROT = cfg.get("yt_rot", 7)
    DEFER = cfg.get("defer", 1)
    nc = bacc.Bacc("TRN2", target_bir_lowering=False, debug=False)

    xT = nc.dram_tensor("xT", (NCT, 128, TALL), F16, kind="ExternalInput").ap()
    wq = nc.dram_tensor("wq", (NCT, 128, CH), F16, kind="ExternalInput").ap()
    wk = nc.dram_tensor("wk", (NCT, 128, CH), F16, kind="ExternalInput").ap()
    wv = nc.dram_tensor("wv", (NCT, 128, CH), F16, kind="ExternalInput").ap()
    bq = nc.dram_tensor("bq", (CH, 1), F32, kind="ExternalInput").ap()
    bk = nc.dram_tensor("bk", (CH, 1), F32, kind="ExternalInput").ap()
    bv = nc.dram_tensor("bv", (1, CH), F32, kind="ExternalInput").ap()
    wproj = nc.dram_tensor("wproj", (CH, C), F16, kind="ExternalInput").ap()
    maskd = nc.dram_tensor("mask", (128, 128), F16, kind="ExternalInput").ap()
    out = nc.dram_tensor("out", (TALL, C), F16, kind="ExternalOutput").ap()

    with tile.TileContext(nc) as tc:
        with (
            tc.tile_pool(name="singles", bufs=1) as singles,
            tc.tile_pool(name="xpool", bufs=3) as xpool,
            tc.tile_pool(name="estp", bufs=EST_B) as estp,
            tc.tile_pool(name="normp", bufs=8) as normp,
            tc.tile_pool(name="ytp", bufs=6) as ytp,
            tc.tile_pool(name="ysp", bufs=6) as ysp,
            tc.tile_pool(name="outp", bufs=4) as outp,
            tc.tile_pool(name="ps_mm", bufs=MM_B, space="PSUM") as ps_mm,
            tc.tile_pool(name="ps_st", bufs=ST_B, space="PSUM") as ps_st,
            tc.tile_pool(name="ps_yt", bufs=1, space="PSUM") as ps_yt,
        ):
            # ---- constants / weights in SBUF ----
            # prefetch the first x chunk before the weights so PE starts early
            xc0 = xpool.tile([128, NCT, TCH], F16, tag="xc", name="xc0")
            nc.sync.dma_start(xc0[:, 0, :], xT[0, :, 0:TCH])
            wq_sb = singles.tile([128, NCT, CH], F16, tag="wq")
            wk_sb = singles.tile([128, NCT, CH], F16, tag="wk")
            wv_sb = singles.tile([128, NCT, CH], F16, tag="wv")
            # weights go down the SWDGE queue, in parallel with the x loads
            nc.gpsimd.dma_start(wq_sb[:], wq.rearrange("ct p m -> p ct m"))
            nc.gpsimd.dma_start(wk_sb[:], wk.rearrange("ct p m -> p ct m"))
            nc.gpsimd.dma_start(wv_sb[:], wv.rearrange("ct p m -> p ct m"))
            for ct0 in range(1, NCT):
                nc.sync.dma_start(xc0[:, ct0, :], xT[ct0, :, 0:TCH])
            bq_sb = singles.tile([CH, 1], F32, tag="bq")
            bk_sb = singles.tile([CH, 1], F32, tag="bk")
            bvb_sb = singles.tile([128, CH], F32, tag="bvb")
            nc.sync.dma_start(bq_sb[:], bq)
            nc.sync.dma_start(bk_sb[:], bk)
            nc.gpsimd.dma_start(
                bvb_sb[:],
                bass.AP(tensor=bv.tensor, offset=bv.offset,
                        ap=[[0, 128]] + list(bv.ap[1:])),
            )
            wproj_sb = singles.tile([CH, C], F16, tag="wpr")
            nc.gpsimd.dma_start(wproj_sb[:], wproj)
            # multiplicative causal mask (1 keep / 0 drop), applied to est
            # after exp on DVE (so exp never waits another engine and the
            # S-psum slots recycle at exp's native rate)
            mask_sb = singles.tile([128, 128], F16, tag="mask")
            nc.gpsimd.dma_start(mask_sb[:], maskd)

            # per-chunk activations
            qT_c = [singles.tile([CH, TCH], F16, tag=f"qT{i}", name=f"qT{i}")
                    for i in range(NCHUNK)]
            kT_c = [singles.tile([CH, TCH], F16, tag=f"kT{i}", name=f"kT{i}")
                    for i in range(NCHUNK)]
            # v layout per chunk: [k-part, k-tile-in-chunk, head, 65]
            v_c = [singles.tile([128, 4, HPC, D + 1], F16, tag=f"v{i}",
                                name=f"v{i}")
                   for i in range(NCHUNK)]
            for i in range(NCHUNK):
                nc.vector.memset(v_c[i][:, :, :, D : D + 1], 1.0)
            # rotating AV accumulators: 7 x [128, 65] fp32 in one PSUM bank
            yt_ps = ps_yt.tile([128, YT_ROT, D + 1], F32, tag="ytp")

            for _rep in range(repeat):
                av_map = {}      # (g, h, j) -> (est_tile, off, qoff)
                cell_n = [0]     # rotation counter for yt_ps

                # ---- phase A: qkv projection of one 512-token chunk ----
                xc_tiles = {}

                def emit_x_dma(chunk):
                    t0 = chunk * TCH
                    if chunk == 0 and _rep == 0:
                        xc_tiles[chunk] = xc0
                        return
                    xc = xpool.tile([128, NCT, TCH], F16, tag="xc")
                    nc.sync.dma_start(
                        xc[:],
                        xT[:, :, t0 : t0 + TCH].rearrange("ct p t -> p ct t"))
                    xc_tiles[chunk] = xc

                def qkv_unit_thunks(chunk):
                    """qkv split into 6 units (q, k, v x4) so the weaver can
                    pace them between S tiles. Returns [(thunk, cost_ns)]."""
                    def qk_unit(w_sb, b_sb, dst):
                        def thunk():
                            xc = xc_tiles[chunk]
                            ps = ps_mm.tile([128, TCH], F32, tag="mm")
                            for ct in range(NCT):
                                nc.tensor.matmul(
                                    ps[:], w_sb[:, ct, :], xc[:, ct, :],
                                    start=(ct == 0), stop=(ct == NCT - 1),
                                )
                            nc.scalar.activation(
                                dst[:], ps[:],
                                mybir.ActivationFunctionType.Identity,
                                bias=b_sb[:],
                            )
                        return thunk

                    def v_unit(s):
                        def thunk():
                            xc = xc_tiles[chunk]
                            psv = ps_mm.tile([128, 128], F32, tag="mm")
                            for ct in range(NCT):
                                nc.tensor.matmul(
                                    psv[:],
                                    xc[:, ct, s * 128 : (s + 1) * 128],
                                    wv_sb[:, ct, :],
                                    start=(ct == 0), stop=(ct == NCT - 1),
                                )
                            nc.vector.tensor_tensor(
                                v_c[chunk][:, s, :, 0:D],
                                psv.rearrange("p (h d) -> p h d", h=HPC),
                                bvb_sb.rearrange("p (h d) -> p h d", h=HPC),
                                op=mybir.AluOpType.add,
                            )
                        return thunk

                    units = [(qk_unit(wq_sb, bq_sb, qT_c[chunk]), 1750),
                             (qk_unit(wk_sb, bk_sb, kT_c[chunk]), 1750)]
                    units += [(v_unit(s), 450) for s in range(TCH // 128)]
                    return units

                def emit_qkv(chunk):
                    emit_x_dma(chunk)
                    for th, _ in qkv_unit_thunks(chunk):
                        th()

                # ---- phase B: S + mask + exp, one ST-tile thunk at a time ----
                def s_tile_thunks(g, h):
                    """Per-ST-tile emission thunks so the top level can weave
                    PE filler work between tiles (exp runs ~2x slower than the
                    S matmuls that feed it)."""
                    b, c = divmod(g, NQC)
                    hb = h * D
                    qTh = qT_c[g][hb : hb + D, :]
                    jw = [(j, 0, 512) for j in range(4 * c + 1)]
                    jw += [(4 * c + 1, 128, 384), (4 * c + 3, 384, 128),
                           (4 * c + 2, 256, 256)]
                    groups = []
                    cur = []
                    fill = 0
                    for j, qoff, w in jw:
                        if cur and fill + w > ST_W:
                            groups.append(cur)
                            cur = []
                            fill = 0
                        cur.append((j, fill, qoff, w))
                        fill += w
                    groups.append(cur)

                    def make(group):
                        def thunk():
                            st = ps_st.tile([128, ST_W], F32, tag="st")
                            est = estp.tile([128, ST_W], F16, tag="est")
                            fill = 0
                            for j, off, qoff, w in group:
                                kTh_j = kT_c[b * NQC + j // 4][
                                    hb : hb + D,
                                    (j % 4) * 128 : (j % 4 + 1) * 128]
                                nc.tensor.matmul(
                                    st[:, off : off + w], kTh_j,
                                    qTh[:, qoff:512],
                                    start=True, stop=True,
                                )
                                av_map[(g, h, j)] = (est, off, qoff)
                                fill = off + w
                            nc.scalar.activation(
                                est[:, 0:fill], st[:, 0:fill],
                                mybir.ActivationFunctionType.Exp,
                                scale=1.0 / np.sqrt(D),
                            )
                            for j, off, qoff, w in group:
                                if j >= 4 * c:  # diagonal: causal mask
                                    nc.gpsimd.tensor_mul(
                                        est[:, off : off + 128],
                                        est[:, off : off + 128],
                                        mask_sb[:],
                                    )
                        return thunk
                    return [make(gr) for gr in groups]

                # ---- phase C: AV for one (chunk, head, q-tile) ----
                def emit_AV(g, h, i):
                    b, c = divmod(g, NQC)
                    qi = 4 * c + i
                    n = cell_n[0] % YT_ROT
                    cell_n[0] += 1
                    yt = yt_ps[:, n, :]
                    for j in range(qi + 1):
                        est, off, qoff = av_map[(g, h, j)]
                        col = off + i * 128 - qoff
                        nc.tensor.matmul(
                            yt, est[:, col : col + 128],
                            v_c[b * NQC + j // 4][:, j % 4, h, :],
                            start=(j == 0), stop=(j == qi),
                        )
                    return yt

                # ---- phase D: normalize one (g, h, i) cell; transpose when
                # both heads are in ----
                def emit_norm_cell(g, h, i, yt):
                    rcol = normp.tile([128, 1], F32, tag="rc")
                    nc.vector.reciprocal(rcol[:], yt[:, D : D + 1])
                    if h == 0:
                        yT = ytp.tile([128, CH], F16, tag="yT")
                        yT_tiles[(g, i)] = yT
                    else:
                        yT = yT_tiles[(g, i)]
                    nc.vector.tensor_scalar_mul(
                        yT[:, h * D : (h + 1) * D], yt[:, 0:D], rcol[:])
                    if h == HPC - 1:
                        ysb = ysp.tile([128, 128], F16, tag="ysb")
                        ysb_tiles[(g, i)] = ysb
                        nc.sync.dma_start_transpose(ysb[:], yT[:])

                def emit_proj_cell(g, i):
                    ysb = ysb_tiles.pop((g, i))
                    tt = g * 4 + i
                    ot = outp.tile([128, 1024], F16, tag="ot")
                    for half in range(2):
                        pso = ps_mm.tile([128, 512], F32, tag="mm")
                        nc.tensor.matmul(
                            pso[:], ysb[:],
                            wproj_sb[:, half * 512 : (half + 1) * 512],
                            start=True, stop=True,
                        )
                        nc.vector.tensor_copy(
                            ot[:, half * 512 : (half + 1) * 512], pso[:])
                    nc.scalar.dma_start(
                        out[tt * 128 : (tt + 1) * 128, :], ot[:])

                yT_tiles = {}
                ysb_tiles = {}
                pend_proj = []   # (g, i) cells whose transpose has been issued

                def av_norm_thunk(g, h, i):
                    def thunk():
                        yt = emit_AV(g, h, i)
                        emit_norm_cell(g, h, i, yt)
                        if h == HPC - 1:
                            pend_proj.append((g, i))
                    return thunk

                # ---- interleaved emission. Attention chunks run in ao[]
                # order (last is a cheap c=0 chunk so the pipeline tail is
                # short). S tiles are the paced stream: between consecutive
                # S tiles the weaver emits ~S_GAP_NS of PE filler work from a
                # global FIFO (proj of transposed cells, qkv units of coming
                # chunks, AV+norm of the previous chunk) so the exp pipe
                # (which runs ~2x slower than the S matmuls) never makes an
                # S matmul park at the head of PE's 4-deep wait queue.
                if "att" in phases:
                    S_GAP_NS = cfg.get("s_gap_ns", 1150)
                    ao = [0, 1, 2, 3, 5, 6, 7, 4]
                    qkv_at = {0: [1], 1: [2], 2: [3], 3: [4, 5], 4: [6],
                              5: [7]}
                    fq = []  # global filler FIFO: (thunk, est_cost_ns)

                    def drain(target_ns):
                        acc = 0
                        while fq and acc < target_ns:
                            th, cost = fq.pop(0)
                            th()
                            acc += cost
                        return acc

                    PROJ_RESERVE = cfg.get("proj_reserve", 0)
                    emit_qkv(0)
                    for it in range(NCHUNK + 2):
                        if "proj" in phases:
                            # hold back a reserve of proj cells as filler for
                            # the late iterations, which have no qkv left
                            keep = PROJ_RESERVE if it < NCHUNK - 2 else 0
                            while len(pend_proj) > keep:
                                gg, ii = pend_proj.pop(0)
                                fq.append((
                                    (lambda a, b: lambda: emit_proj_cell(a, b)
                                     )(gg, ii), 600))
                        if "qkv" in phases and it < NCHUNK:
                            for ch in qkv_at.get(it, []):
                                emit_x_dma(ch)
                                fq.extend(qkv_unit_thunks(ch))
                        if 1 <= it <= NCHUNK:
                            gp = ao[it - 1]
                            for h in range(HPC):
                                for i in range(4):
                                    b, c = divmod(gp, NQC)
                                    cost = (4 * c + i + 1) * 30 + 450
                                    fq.append((av_norm_thunk(gp, h, i), cost))
                        if it < NCHUNK:
                            for sth in s_tile_thunks(ao[it], 0) + \
                                    s_tile_thunks(ao[it], 1):
                                drain(S_GAP_NS)
                                sth()
                        else:
                            drain(10**9)
                    drain(10**9)
                elif "qkv" in phases:
                    for g in range(1, NCHUNK):
                        emit_qkv(g)

    nc.compile()
    return nc


_NC_CACHE = None


def _get_nc():
    global _NC_CACHE
    if _NC_CACHE is None:
        _NC_CACHE = build_nc()
    return _NC_CACHE


def make_in_maps(x, Wqkv, bqkv, bproj=None):
    x = np.asarray(x, dtype=np.float32)
    Wqkv = np.asarray(Wqkv, dtype=np.float32)
    bqkv = np.asarray(bqkv, dtype=np.float32)

    x_flat = x.reshape(TALL, C)
    xT = np.ascontiguousarray(x_flat.T).reshape(NCT, 128, TALL).astype(np.float16)
    mask = np.triu(np.ones((128, 128), dtype=np.float16))  # mask[k,q]=1 iff k<=q

    in_maps = []
    for i in range(N_CORES):
        cs = slice(i * CH, (i + 1) * CH)
        ks = slice(C + i * CH, C + (i + 1) * CH)
        vs = slice(2 * C + i * CH, 2 * C + (i + 1) * CH)
        in_maps.append({
            "xT": xT,
            "wq": np.ascontiguousarray(Wqkv[:, cs]).reshape(NCT, 128, CH)
                    .astype(np.float16),
            "wk": np.ascontiguousarray(Wqkv[:, ks]).reshape(NCT, 128, CH)
                    .astype(np.float16),
            "wv": np.ascontiguousarray(Wqkv[:, vs]).reshape(NCT, 128, CH)
                    .astype(np.float16),
            "bq": np.ascontiguousarray(bqkv[cs]).reshape(CH, 1)
                    .astype(np.float32),
            "bk": np.ascontiguousarray(bqkv[ks]).reshape(CH, 1)
                    .astype(np.float32),
            "bv": np.ascontiguousarray(bqkv[vs]).reshape(1, CH)
                    .astype(np.float32),
            "mask": mask,
        })
    return in_maps


def kernel(x, Wqkv, bqkv, Wproj, bproj, _trace=False, _trace_kwargs=None):
    Wproj = np.asarray(Wproj, dtype=np.float32)
    bproj = np.asarray(bproj, dtype=np.float32)
    nc = _get_nc()
    in_maps = make_in_maps(x, Wqkv, bqkv)
    for i in range(N_CORES):
        in_maps[i]["wproj"] = np.ascontiguousarray(
            Wproj[i * CH : (i + 1) * CH, :]).astype(np.float16)
    res = run_bass_kernel_spmd(
        nc, in_maps, core_ids=list(range(N_CORES)),
        trace=_trace, **(_trace_kwargs or {}),
    )
    acc = res.results[0]["out"].astype(np.float32).copy()
    for c in range(1, N_CORES):
        acc += res.results[c]["out"]
    acc += bproj.reshape(1, C)
    out = acc.reshape(B, T, C)
    if _trace:
        return out, res
    return out


# revision 55
# speedup vs baseline: 1.1422x; 1.1422x over previous
"""Causal self-attention (B=2, T=2048, C=1024, H=16) on 8 trn2 NeuronCores.

Sharding: tensor-parallel over heads. Each core owns 2 heads (128 channels):
  - qkv projection for its 128 q/k/v columns (full x, transposed layout xT)
  - causal attention for its 2 heads x 2 batches
  - output projection rows for its 128 channels -> partial (4096, 1024) output
Host sums the 8 partial outputs (the "all-reduce") and adds bproj once.

All matmul inputs are fp16 (activations + weights; PSUM accumulates fp32):
fp16 moves 1 col/cycle at any width (f32r drops to 1/4 rate under 256 cols)
and halves HBM traffic. Softmax skips max-subtraction (logits ~ N(0,1)).

Structure per 512-token chunk:
  qkv: q,k wide [ch, t] (stationary w-tile, moving x); v natural [t, ch]
       (stationary x-tile, moving w) so no PE transpose is needed; biases are
       folded into the PSUM->SBUF copies on the Pool engine.
  S:   st[k, q] tiles (stationary kT slice, moving qT), exp on ACT into fp16
       est tiles, causal diag masked post-exp on DVE.
  AV:  est [k, q-128] slices as STATIONARY, moving v_aug [k, 65] (64 v cols +
       ones col) -> yt [q, 65] PSUM; col 64 accumulates the softmax
       denominator per-partition, so normalization is one reciprocal + one
       per-partition tensor_scalar (DVE), no broadcast matmuls.
  y:   merged [t, ch] tile -> DMA-transpose (xbar) -> y [ch, t] fp16.
  proj: stationary y-tile, moving Wproj -> PSUM f32 -> DMA straight to DRAM
       (partial sums; bias added on host).
"""

import sys

if "/opt/trn_rl_repo" not in sys.path:
    sys.path.insert(0, "/opt/trn_rl_repo")

import numpy as np

import concourse.bass as bass
import concourse.mybir as mybir
import concourse.tile as tile
from concourse import bacc
from concourse.bass_utils import run_bass_kernel_spmd

# Problem shape (hardcoded per contest contract)
B, T, C, H = 2, 2048, 1024, 16
D = C // H                # 64 head dim
N_CORES = 8
HPC = H // N_CORES        # 2 heads per core
CH = HPC * D              # 128 channels per core
TALL = B * T              # 4096 flattened tokens
NCT = C // 128            # 8 contraction tiles
TCH = 512                 # t-chunk
NCHUNK = TALL // TCH      # 8
NQC = T // 512            # 4 q-chunks per batch
F32 = mybir.dt.float32
F16 = mybir.dt.float16


def build_nc(phases=("qkv", "att", "proj"), repeat=1, cfg=None):
    cfg = cfg or {}
    ST_W = cfg.get("st_w", 1024)
    ST_B = cfg.get("st_bufs", 2)
    MM_B = cfg.get("mm_bufs", 3)
    EST_B = cfg.get("est_bufs", 28)
    YT_ROT = cfg.get("yt_rot", 7)
    DEFER = cfg.get("defer", 1)
    nc = bacc.Bacc("TRN2", target_bir_lowering=False, debug=False)

    xT = nc.dram_tensor("xT", (NCT, 128, TALL), F16, kind="ExternalInput").ap()
    wq = nc.dram_tensor("wq", (128, NCT, CH), F16, kind="ExternalInput").ap()
    wk = nc.dram_tensor("wk", (128, NCT, CH), F16, kind="ExternalInput").ap()
    wv = nc.dram_tensor("wv", (128, NCT, CH), F16, kind="ExternalInput").ap()
    bq = nc.dram_tensor("bq", (CH, 1), F32, kind="ExternalInput").ap()
    bk = nc.dram_tensor("bk", (CH, 1), F32, kind="ExternalInput").ap()
    bv = nc.dram_tensor("bv", (1, CH), F32, kind="ExternalInput").ap()
    wproj = nc.dram_tensor("wproj", (CH, C), F16, kind="ExternalInput").ap()
    maskd = nc.dram_tensor("mask", (128, 128), F16, kind="ExternalInput").ap()
    out = nc.dram_tensor("out", (TALL, C), F16, kind="ExternalOutput").ap()

    with tile.TileContext(nc) as tc:
        with (
            tc.tile_pool(name="singles", bufs=1) as singles,
            tc.tile_pool(name="xpool", bufs=3) as xpool,
            tc.tile_pool(name="estp", bufs=EST_B) as estp,
            tc.tile_pool(name="normp", bufs=8) as normp,
            tc.tile_pool(name="ytp", bufs=6) as ytp,
            tc.tile_pool(name="ysp", bufs=6) as ysp,
            tc.tile_pool(name="outp", bufs=cfg.get("out_bufs", 4)) as outp,
            tc.tile_pool(name="ps_mm", bufs=MM_B, space="PSUM") as ps_mm,
            tc.tile_pool(name="ps_st", bufs=ST_B, space="PSUM") as ps_st,
            tc.tile_pool(name="ps_yt", bufs=1, space="PSUM") as ps_yt,
        ):
            # ---- constants / weights in SBUF ----
            # prefetch the first x chunk before the weights so PE starts early
            # staging order: wq first on sync (gates the first matmul), the
            # rest split across the two HWDGE queues. Weights are host-side
            # pre-transposed to [p, ct, m] so rows are 2KB-contiguous.
            xc0 = xpool.tile([128, NCT, TCH], F16, tag="xc", name="xc0")
            wq_sb = singles.tile([128, NCT, CH], F16, tag="wq")
            wk_sb = singles.tile([128, NCT, CH], F16, tag="wk")
            wv_sb = singles.tile([128, NCT, CH], F16, tag="wv")
            nc.sync.dma_start(wq_sb[:], wq)
            nc.sync.dma_start(xc0[:, 0, :], xT[0, :, 0:TCH])
            nc.scalar.dma_start(wk_sb[:], wk)
            nc.scalar.dma_start(wv_sb[:], wv)
            for ct0 in range(1, NCT):
                nc.sync.dma_start(xc0[:, ct0, :], xT[ct0, :, 0:TCH])
            bq_sb = singles.tile([CH, 1], F32, tag="bq")
            bk_sb = singles.tile([CH, 1], F32, tag="bk")
            bvb_sb = singles.tile([128, CH], F32, tag="bvb")
            nc.scalar.dma_start(bq_sb[:], bq)
            nc.scalar.dma_start(bk_sb[:], bk)
            nc.scalar.dma_start(
                bvb_sb[:],
                bass.AP(tensor=bv.tensor, offset=bv.offset,
                        ap=[[0, 128]] + list(bv.ap[1:])),
            )
            wproj_sb = singles.tile([CH, C], F16, tag="wpr")
            nc.scalar.dma_start(wproj_sb[:], wproj)
            # multiplicative causal mask (1 keep / 0 drop), applied to est
            # after exp on Pool (so exp never waits another engine and the
            # S-psum slots recycle at exp's native rate)
            mask_sb = singles.tile([128, 128], F16, tag="mask")
            nc.scalar.dma_start(mask_sb[:], maskd)

            # per-chunk activations
            qT_c = [singles.tile([CH, TCH], F16, tag=f"qT{i}", name=f"qT{i}")
                    for i in range(NCHUNK)]
            kT_c = [singles.tile([CH, TCH], F16, tag=f"kT{i}", name=f"kT{i}")
                    for i in range(NCHUNK)]
            # v layout per chunk: [k-part, k-tile-in-chunk, head, 65]
            v_c = [singles.tile([128, 4, HPC, D + 1], F16, tag=f"v{i}",
                                name=f"v{i}")
                   for i in range(NCHUNK)]
            for i in range(NCHUNK):
                nc.vector.memset(v_c[i][:, :, :, D : D + 1], 1.0)
            # rotating AV accumulators: 7 x [128, 65] fp32 in one PSUM bank
            yt_ps = ps_yt.tile([128, YT_ROT, D + 1], F32, tag="ytp")

            for _rep in range(repeat):
                av_map = {}      # (g, h, j) -> (est_tile, off, qoff)
                cell_n = [0]     # rotation counter for yt_ps

                # ---- phase A: qkv projection of one 512-token chunk ----
                xc_tiles = {}

                def emit_x_dma(chunk):
                    t0 = chunk * TCH
                    if chunk == 0 and _rep == 0:
                        xc_tiles[chunk] = xc0
                        return
                    xc = xpool.tile([128, NCT, TCH], F16, tag="xc")
                    nc.sync.dma_start(
                        xc[:],
                        xT[:, :, t0 : t0 + TCH].rearrange("ct p t -> p ct t"))
                    xc_tiles[chunk] = xc

                def qkv_unit_thunks(chunk):
                    """qkv split into 6 units (q, k, v x4) so the weaver can
                    pace them between S tiles. Returns [(thunk, cost_ns)]."""
                    def qk_unit(w_sb, b_sb, dst):
                        def thunk():
                            xc = xc_tiles[chunk]
                            ps = ps_mm.tile([128, TCH], F32, tag="mm")
                            for ct in range(NCT):
                                nc.tensor.matmul(
                                    ps[:], w_sb[:, ct, :], xc[:, ct, :],
                                    start=(ct == 0), stop=(ct == NCT - 1),
                                )
                            nc.vector.tensor_scalar_add(dst[:], ps[:], b_sb[:])
                        return thunk

                    def v_unit(s):
                        def thunk():
                            xc = xc_tiles[chunk]
                            psv = ps_mm.tile([128, 128], F32, tag="mm")
                            for ct in range(NCT):
                                nc.tensor.matmul(
                                    psv[:],
                                    xc[:, ct, s * 128 : (s + 1) * 128],
                                    wv_sb[:, ct, :],
                                    start=(ct == 0), stop=(ct == NCT - 1),
                                )
                            nc.vector.tensor_tensor(
                                v_c[chunk][:, s, :, 0:D],
                                psv.rearrange("p (h d) -> p h d", h=HPC),
                                bvb_sb.rearrange("p (h d) -> p h d", h=HPC),
                                op=mybir.AluOpType.add,
                            )
                        return thunk

                    units = [(qk_unit(wq_sb, bq_sb, qT_c[chunk]), 1750),
                             (qk_unit(wk_sb, bk_sb, kT_c[chunk]), 1750)]
                    units += [(v_unit(s), 450) for s in range(TCH // 128)]
                    return units

                def emit_qkv(chunk):
                    emit_x_dma(chunk)
                    for th, _ in qkv_unit_thunks(chunk):
                        th()

                # ---- phase B: S + mask + exp, one ST-tile thunk at a time ----
                def s_tile_thunks(g, h):
                    """Per-ST-tile emission thunks so the top level can weave
                    PE filler work between tiles (exp runs ~2x slower than the
                    S matmuls that feed it)."""
                    b, c = divmod(g, NQC)
                    hb = h * D
                    qTh = qT_c[g][hb : hb + D, :]
                    jw = [(j, 0, 512) for j in range(4 * c + 1)]
                    jw += [(4 * c + 1, 128, 384), (4 * c + 3, 384, 128),
                           (4 * c + 2, 256, 256)]
                    groups = []
                    cur = []
                    fill = 0
                    for j, qoff, w in jw:
                        if cur and fill + w > ST_W:
                            groups.append(cur)
                            cur = []
                            fill = 0
                        cur.append((j, fill, qoff, w))
                        fill += w
                    groups.append(cur)

                    def make(group):
                        def thunk():
                            st = ps_st.tile([128, ST_W], F32, tag="st")
                            est = estp.tile([128, ST_W], F16, tag="est")
                            fill = 0
                            for j, off, qoff, w in group:
                                kTh_j = kT_c[b * NQC + j // 4][
                                    hb : hb + D,
                                    (j % 4) * 128 : (j % 4 + 1) * 128]
                                nc.tensor.matmul(
                                    st[:, off : off + w], kTh_j,
                                    qTh[:, qoff:512],
                                    start=True, stop=True,
                                )
                                av_map[(g, h, j)] = (est, off, qoff)
                                fill = off + w
                            nc.scalar.activation(
                                est[:, 0:fill], st[:, 0:fill],
                                mybir.ActivationFunctionType.Exp,
                                scale=1.0 / np.sqrt(D),
                            )
                            for j, off, qoff, w in group:
                                if j >= 4 * c:  # diagonal: causal mask
                                    nc.gpsimd.tensor_mul(
                                        est[:, off : off + 128],
                                        est[:, off : off + 128],
                                        mask_sb[:],
                                    )
                        return thunk
                    return [make(gr) for gr in groups]

                # ---- phase C: AV for one (chunk, head, q-tile) ----
                def emit_AV(g, h, i):
                    b, c = divmod(g, NQC)
                    qi = 4 * c + i
                    n = cell_n[0] % YT_ROT
                    cell_n[0] += 1
                    yt = yt_ps[:, n, :]
                    for j in range(qi + 1):
                        est, off, qoff = av_map[(g, h, j)]
                        col = off + i * 128 - qoff
                        nc.tensor.matmul(
                            yt, est[:, col : col + 128],
                            v_c[b * NQC + j // 4][:, j % 4, h, :],
                            start=(j == 0), stop=(j == qi),
                        )
                    return yt

                # ---- phase D: normalize one (g, h, i) cell; transpose when
                # both heads are in ----
                def emit_norm_cell(g, h, i, yt):
                    rcol = normp.tile([128, 1], F32, tag="rc")
                    nc.vector.reciprocal(rcol[:], yt[:, D : D + 1])
                    if h == 0:
                        yT = ytp.tile([128, CH], F16, tag="yT")
                        yT_tiles[(g, i)] = yT
                    else:
                        yT = yT_tiles[(g, i)]
                    nc.vector.tensor_scalar_mul(
                        yT[:, h * D : (h + 1) * D], yt[:, 0:D], rcol[:])
                    if h == HPC - 1:
                        ysb = ysp.tile([128, 128], F16, tag="ysb")
                        ysb_tiles[(g, i)] = ysb
                        nc.sync.dma_start_transpose(ysb[:], yT[:])

                def emit_proj_cell(g, i, act_help=False):
                    ysb = ysb_tiles.pop((g, i))
                    tt = g * 4 + i
                    ot = outp.tile([128, 1024], F16, tag="ot")
                    for half in range(2):
                        pso = ps_mm.tile([128, 512], F32, tag="mm")
                        nc.tensor.matmul(
                            pso[:], ysb[:],
                            wproj_sb[:, half * 512 : (half + 1) * 512],
                            start=True, stop=True,
                        )
                        osl = ot[:, half * 512 : (half + 1) * 512]
                        if act_help and half == 1:
                            # tail chunks: exp is done, ACT has spare cycles
                            nc.scalar.activation(
                                osl, pso[:],
                                mybir.ActivationFunctionType.Identity)
                        else:
                            nc.vector.tensor_copy(osl, pso[:])
                    eng = nc.scalar if tt % 2 else nc.sync
                    eng.dma_start(out[tt * 128 : (tt + 1) * 128, :], ot[:])

                yT_tiles = {}
                ysb_tiles = {}
                pend_proj = []   # (g, i) cells whose transpose has been issued

                def av_norm_thunk(g, h, i):
                    def thunk():
                        yt = emit_AV(g, h, i)
                        emit_norm_cell(g, h, i, yt)
                        if h == HPC - 1:
                            pend_proj.append((g, i))
                    return thunk

                # ---- interleaved emission. Attention chunks run in ao[]
                # order (last is a cheap c=0 chunk so the pipeline tail is
                # short). S tiles are the paced stream: between consecutive
                # S tiles the weaver emits ~S_GAP_NS of PE filler work from a
                # global FIFO (proj of transposed cells, qkv units of coming
                # chunks, AV+norm of the previous chunk) so the exp pipe
                # (which runs ~2x slower than the S matmuls) never makes an
                # S matmul park at the head of PE's 4-deep wait queue.
                if "att" in phases:
                    S_GAP_NS = cfg.get("s_gap_ns", 1250)
                    ao = [0, 1, 2, 3, 5, 6, 7, 4]
                    qkv_at = {0: [1], 1: [2], 2: [3], 3: [4, 5], 4: [6],
                              5: [7]}
                    fq = []  # global filler FIFO: (thunk, est_cost_ns)

                    def drain(target_ns):
                        acc = 0
                        while fq and acc < target_ns:
                            th, cost = fq.pop(0)
                            th()
                            acc += cost
                        return acc

                    PROJ_RESERVE = cfg.get("proj_reserve", 0)
                    emit_qkv(0)
                    for it in range(NCHUNK + 2):
                        if "proj" in phases:
                            # hold back a reserve of proj cells as filler for
                            # the late iterations, which have no qkv left
                            keep = PROJ_RESERVE if it < NCHUNK - 2 else 0
                            while len(pend_proj) > keep:
                                gg, ii = pend_proj.pop(0)
                                ah = it >= NCHUNK
                                fq.append((
                                    (lambda a, b, c:
                                     lambda: emit_proj_cell(a, b, act_help=c)
                                     )(gg, ii, ah), 600))
                        if "qkv" in phases and it < NCHUNK:
                            for ch in qkv_at.get(it, []):
                                emit_x_dma(ch)
                                fq.extend(qkv_unit_thunks(ch))
                        if 1 <= it <= NCHUNK:
                            gp = ao[it - 1]
                            b, c = divmod(gp, NQC)
                            for h in range(HPC):
                                for i in range(4):
                                    cost = (4 * c + i + 1) * 30 + 450
                                    fq.append((av_norm_thunk(gp, h, i), cost))
                        if it < NCHUNK:
                            for sth in s_tile_thunks(ao[it], 0) + \
                                    s_tile_thunks(ao[it], 1):
                                drain(S_GAP_NS)
                                sth()
                        else:
                            drain(10**9)
                    drain(10**9)
                elif "qkv" in phases:
                    for g in range(1, NCHUNK):
                        emit_qkv(g)

    nc.compile()
    return nc


_NC_CACHE = None


def _get_nc():
    global _NC_CACHE
    if _NC_CACHE is None:
        _NC_CACHE = build_nc()
    return _NC_CACHE


def make_in_maps(x, Wqkv, bqkv, bproj=None):
    x = np.asarray(x, dtype=np.float32)
    Wqkv = np.asarray(Wqkv, dtype=np.float32)
    bqkv = np.asarray(bqkv, dtype=np.float32)

    x_flat = x.reshape(TALL, C)
    xT = np.ascontiguousarray(x_flat.T).reshape(NCT, 128, TALL).astype(np.float16)
    mask = np.triu(np.ones((128, 128), dtype=np.float16))  # mask[k,q]=1 iff k<=q

    in_maps = []
    for i in range(N_CORES):
        cs = slice(i * CH, (i + 1) * CH)
        ks = slice(C + i * CH, C + (i + 1) * CH)
        vs = slice(2 * C + i * CH, 2 * C + (i + 1) * CH)
        in_maps.append({
            "xT": xT,
            "wq": np.ascontiguousarray(
                Wqkv[:, cs].reshape(NCT, 128, CH).transpose(1, 0, 2)
            ).astype(np.float16),
            "wk": np.ascontiguousarray(
                Wqkv[:, ks].reshape(NCT, 128, CH).transpose(1, 0, 2)
            ).astype(np.float16),
            "wv": np.ascontiguousarray(
                Wqkv[:, vs].reshape(NCT, 128, CH).transpose(1, 0, 2)
            ).astype(np.float16),
            "bq": np.ascontiguousarray(bqkv[cs]).reshape(CH, 1)
                    .astype(np.float32),
            "bk": np.ascontiguousarray(bqkv[ks]).reshape(CH, 1)
                    .astype(np.float32),
            "bv": np.ascontiguousarray(bqkv[vs]).reshape(1, CH)
                    .astype(np.float32),
            "mask": mask,
        })
    return in_maps


def kernel(x, Wqkv, bqkv, Wproj, bproj, _trace=False, _trace_kwargs=None):
    Wproj = np.asarray(Wproj, dtype=np.float32)
    bproj = np.asarray(bproj, dtype=np.float32)
    nc = _get_nc()
    in_maps = make_in_maps(x, Wqkv, bqkv)
    for i in range(N_CORES):
        in_maps[i]["wproj"] = np.ascontiguousarray(
            Wproj[i * CH : (i + 1) * CH, :]).astype(np.float16)
    res = run_bass_kernel_spmd(
        nc, in_maps, core_ids=list(range(N_CORES)),
        trace=_trace, **(_trace_kwargs or {}),
    )
    acc = res.results[0]["out"].astype(np.float32).copy()
    for c in range(1, N_CORES):
        acc += res.results[c]["out"]
    acc += bproj.reshape(1, C)
    out = acc.reshape(B, T, C)
    if _trace:
        return out, res
    return out
